# revision 12
# baseline (speedup 1.0000x reference)
"""Trainium2 Bass kernel for DenseBlock: sync-BN (training stats) + binarized
3x3 conv + dense concat.

Reference computation (shapes hardcoded):
  x: (32, 256, 56, 56) f32
  mean/var over (N,H,W) per channel  ->  xn = (x-mean)*rsqrt(var+eps)*gamma+beta
  out_conv = conv3x3(xn, sign(w)) + b      (padding=1)
  return concat([x, out_conv], axis=1)     -> (32, 320, 56, 56)

Distribution: data-parallel over batch (4 images per core, 8 cores),
weights replicated, sync-BN via an on-device AllReduce of per-core
(sum, sumsq) partials.

Device layout per core:
  - x is host-padded to W=64 (cols 56..63 zero) so each row is a 64-element
    stride; each (ktile, image) lives in SBUF as [128p, 60, 64]: rows 0-1 and
    58-59 are zero padding, the image occupies rows 2..57. With this layout
    every 3x3 tap's input window is the SAME [8, 56] pattern shifted by
    dh*64 + dw elements, always reading in-bounds (pad rows/cols supply the
    conv zero padding exactly).
  - bn_stats/bn_aggr one-pass stats over the image cols 0..55 ->
    (sum, sumsq) -> 2KB AllReduce -> per-channel scale s, shift t
  - xn = s*x + t in place on image cols (kt0 on ACT, kt1 on DVE)
  - conv: per output tile (image n, 8-row block) the 9 taps x 2 K-tiles are
    18 matmuls, each writing the full [64, 8, 56] psum footprint (uniform
    accumulation group). The two K-tiles (C=256 -> 2x128) run CONCURRENTLY
    in the two 64-column halves of the PE array (col-tiling, M=64 each),
    psum partitions [0:64] / [64:128].
  - epilogue: out = (psum_lo + b) + psum_hi in one DVE op, DMA out
  - host concatenates raw x with the gathered conv outputs
"""

import os
import sys
from contextlib import ExitStack

import numpy as np

sys.path.insert(0, "/opt/trn_rl_repo")

from concourse import bacc, bass, mybir, tile  # noqa: E402
from concourse.bass_utils import run_bass_kernel_spmd  # noqa: E402

N, C, H, W, O = 32, 256, 56, 56, 64
NCORES = 8
NPER = N // NCORES  # 4 images per core
KT = 2  # channel tiles of 128
PIX = H * W  # 3136
EPS = 1e-5
HB = 8  # psum tile height (8 rows x 56 = 448 <= 512 f32 psum bank)
WP = 64  # host-padded row width
NHB = H // HB  # 7
TOP = 2  # top pad rows in the sbuf tile
ROWS = TOP + H + 2  # 60
F32 = mybir.dt.float32

TAPS = [(dh, dw) for dh in (-1, 0, 1) for dw in (-1, 0, 1)]


def shifted(ap, delta: int):
    """Same access pattern, element offset shifted by delta."""
    return bass.AP(
        tensor=ap.tensor, offset=ap.offset + delta, ap=ap.ap
    )


def build_program(variant: str | None = None) -> bacc.Bacc:
    """variant: 'coltile' (default) runs the two K-tiles concurrently in the
    two column halves of the PE array; 'serial' accumulates all 18 matmuls
    into one [64, ...] psum tile."""
    if variant is None:
        variant = os.environ.get("BASS_VARIANT", "coltile")
    coltile = variant == "coltile"

    nc = bacc.Bacc(num_devices=NCORES)
    x_ext = nc.declare_dram_parameter("x", [NPER, C, H, WP], F32, isOutput=False)
    w_ext = nc.declare_dram_parameter("wbt", [128, KT, 9, O], F32, isOutput=False)
    g_ext = nc.declare_dram_parameter("gamma2", [128, KT], F32, isOutput=False)
    be_ext = nc.declare_dram_parameter("beta2", [128, KT], F32, isOutput=False)
    b_ext = nc.declare_dram_parameter("bvec", [O, 1], F32, isOutput=False)
    out_ext = nc.declare_dram_parameter("out", [NPER, O, H, W], F32, isOutput=True)

    with tile.TileContext(nc) as tc, ExitStack() as ctx:
        xpool = ctx.enter_context(tc.tile_pool(name="x", bufs=1))
        cpool = ctx.enter_context(tc.tile_pool(name="consts", bufs=1))
        spool = ctx.enter_context(tc.tile_pool(name="stats", bufs=1))
        pspool = ctx.enter_context(
            tc.tile_pool(name="psum", bufs=4, space=bass.MemorySpace.PSUM)
        )
        opool = ctx.enter_context(tc.tile_pool(name="ob", bufs=4))
        dpool = ctx.enter_context(tc.tile_pool(name="dram", bufs=1, space="DRAM"))

        # x shard: one tile per (channel-tile, image); image rows at [2:58]
        xk = [
            [xpool.tile([128, ROWS, WP], F32, tag=f"xk{k}_{n}", name=f"xk{k}_{n}")
             for n in range(NPER)]
            for k in range(KT)
        ]
        w_sb = cpool.tile([128, KT, 9, O], F32, tag="w", name="w_sb")
        g_sb = cpool.tile([128, KT], F32, tag="g", name="g_sb")
        be_sb = cpool.tile([128, KT], F32, tag="be", name="be_sb")
        b_sb = cpool.tile([O, 1], F32, tag="b", name="b_sb")

        nc.sync.dma_start(out=w_sb[:], in_=w_ext[:])
        nc.sync.dma_start(out=g_sb[:], in_=g_ext[:])
        nc.sync.dma_start(out=be_sb[:], in_=be_ext[:])
        nc.sync.dma_start(out=b_sb[:], in_=b_ext[:])

        for k in range(KT):
            for n in range(NPER):
                t = xk[k][n]
                nc.gpsimd.memset(t[:, 0:TOP, :], 0.0)
                nc.gpsimd.memset(t[:, TOP + H : ROWS, :], 0.0)
                nc.sync.dma_start(
                    out=t[:, TOP : TOP + H, :],
                    in_=x_ext[n, k * 128 : (k + 1) * 128],
                )

        # ---- local batch-norm stats (one pass over x on DVE) ----
        # stats run over the full 64-wide rows INCLUDING the zero pad cols:
        # zeros add nothing to sum/sumsq, so converting (mean, var) with the
        # padded count recovers the exact partial sums.
        bns = spool.tile([128, KT, NPER, NHB, 6], F32, tag="bns", name="bns")
        for k in range(KT):
            for n in range(NPER):
                for ib in range(NHB):
                    r0 = TOP + ib * HB
                    nc.vector.bn_stats(
                        out=bns[:, k, n, ib],
                        in_=xk[k][n][:, r0 : r0 + HB, :].opt(),
                    )
        mv = spool.tile([128, KT, 2], F32, tag="mv", name="mv")  # (mean, var) local
        for k in range(KT):
            nc.vector.bn_aggr(out=mv[:, k], in_=bns[:, k])

        # convert to (sum, sumsq) partials for the cross-core reduction
        part = spool.tile([128, KT, 2], F32, tag="part", name="part")
        tmp = spool.tile([128, KT], F32, tag="tmp", name="tmp")
        npix = float(NPER * H * WP)  # padded count (zeros cancel)
        nc.vector.tensor_scalar_mul(part[:, :, 0], mv[:, :, 0], npix)
        nc.vector.tensor_mul(tmp[:], mv[:, :, 0], mv[:, :, 0])
        nc.vector.tensor_add(tmp[:], tmp[:], mv[:, :, 1])
        nc.vector.tensor_scalar_mul(part[:, :, 1], tmp[:], npix)

        cc_in = dpool.tile([128, KT, 2], F32, tag="ccin", name="cc_in")
        cc_out = dpool.tile(
            [128, KT, 2], F32, tag="ccout", name="cc_out", addr_space="Shared"
        )
        nc.gpsimd.dma_start(out=cc_in[:], in_=part[:])
        nc.gpsimd.collective_compute(
            "AllReduce",
            mybir.AluOpType.add,
            replica_groups=[list(range(NCORES))],
            ins=[cc_in[:].opt()],
            outs=[cc_out[:].opt()],
        )
        gpart = spool.tile([128, KT, 2], F32, tag="gpart", name="gpart")
        nc.gpsimd.dma_start(out=gpart[:], in_=cc_out[:])

        # ---- global scale/shift: s = gamma*rsqrt(var+eps), t = beta - mean*s
        gm = spool.tile([128, KT], F32, tag="gm", name="gm")
        vr = spool.tile([128, KT], F32, tag="vr", name="vr")
        msq = spool.tile([128, KT], F32, tag="msq", name="msq")
        s_sb = spool.tile([128, KT], F32, tag="s", name="s_sb")
        t_sb = spool.tile([128, KT], F32, tag="t", name="t_sb")
        inv_total = 1.0 / float(N * PIX)
        nc.vector.tensor_scalar_mul(gm[:], gpart[:, :, 0], inv_total)
        nc.vector.tensor_scalar_mul(vr[:], gpart[:, :, 1], inv_total)  # E[x^2]
        nc.vector.tensor_mul(msq[:], gm[:], gm[:])
        nc.vector.tensor_sub(vr[:], vr[:], msq[:])  # var
        epst = spool.tile([128, 1], F32, tag="eps", name="epst")
        nc.vector.memset(epst[:], EPS)
        nc.scalar.activation(
            vr[:], vr[:], mybir.ActivationFunctionType.Sqrt, bias=epst[:]
        )  # std
        nc.vector.reciprocal(vr[:], vr[:])  # 1/std
        nc.vector.tensor_mul(s_sb[:], g_sb[:], vr[:])
        nc.vector.tensor_mul(t_sb[:], gm[:], s_sb[:])
        nc.vector.tensor_sub(t_sb[:], be_sb[:], t_sb[:])

        # ---- xn = s*x + t in place on image cols; kt0 on ACT, kt1 on DVE
        for n in range(NPER):
            img0 = xk[0][n][:, TOP : TOP + H, 0:W]
            img1 = xk[1][n][:, TOP : TOP + H, 0:W]
            nc.scalar.activation(
                img0,
                img0,
                mybir.ActivationFunctionType.Identity,
                bias=t_sb[:, 0:1],
                scale=s_sb[:, 0:1],
            )
            nc.vector.tensor_scalar(
                img1,
                img1,
                s_sb[:, 1:2],
                t_sb[:, 1:2],
                mybir.AluOpType.mult,
                mybir.AluOpType.add,
            )

        # ---- conv: 18 uniform matmuls per output tile ----
        # rhs for tap (dh, dw) = the [8, 56] window shifted dh*64+dw elements
        for n in range(NPER):
            for ib in range(NHB):
                r0 = TOP + ib * HB
                base = [xk[k][n][:, r0 : r0 + HB, 0:W] for k in range(KT)]
                if coltile:
                    ps = pspool.tile([128, HB, W], F32, tag="ps", name="ps")
                else:
                    ps = pspool.tile([O, HB, W], F32, tag="ps", name="ps")
                for ti, (dh, dw) in enumerate(TAPS):
                    tap = (dh + 1) * 3 + (dw + 1)
                    for k in range(KT):
                        if coltile:
                            out_ap = ps[64 * k : 64 * k + 64]
                            start = ti == 0
                            stop = ti == len(TAPS) - 1
                        else:
                            out_ap = ps[:]
                            start = ti == 0 and k == 0
                            stop = ti == len(TAPS) - 1 and k == KT - 1
                        nc.tensor.matmul(
                            out_ap,
                            w_sb[:, k, tap, :],
                            shifted(base[k], dh * WP + dw),
                            start=start,
                            stop=stop,
                            # the interp's group-conflict check is partition-
                            # blind; the two col-split halves falsely collide
                            skip_group_check=coltile,
                        )
                ob = opool.tile([O, HB, W], F32, tag="ob", name="ob")
                if coltile:
                    # PSUM reads may cross partitions (SB operands may not):
                    # ACT: ob_hi = psum_hi + b ; DVE: ob = ob_hi + psum_lo
                    ob_hi = opool.tile([O, HB, W], F32, tag="obhi", name="ob_hi")
                    nc.scalar.activation(
                        ob_hi[:],
                        ps[64:128],
                        mybir.ActivationFunctionType.Identity,
                        bias=b_sb[:],
                    )
                    nc.vector.tensor_add(ob[:], ob_hi[:], ps[0:64])
                else:
                    nc.vector.tensor_scalar_add(ob[:], ps[:], b_sb[:])
                nc.sync.dma_start(
                    out=out_ext[n, :, ib * HB : (ib + 1) * HB, :], in_=ob[:]
                )

    nc.finalize()
    return nc


def prep_inputs(x, gamma, beta, w, b):
    """Host-side layout prep. Returns (raw x, per-core input maps)."""
    x = np.ascontiguousarray(np.asarray(x, dtype=np.float32))
    gamma = np.asarray(gamma, dtype=np.float32)
    beta = np.asarray(beta, dtype=np.float32)
    w = np.asarray(w, dtype=np.float32)
    b = np.asarray(b, dtype=np.float32)

    # pad W to 64 with zero columns (the conv's zero padding comes from these)
    xp = np.zeros((N, C, H, WP), dtype=np.float32)
    xp[..., :W] = x

    # sign(w) transposed to [c_local=128, kt, tap, o], contiguous
    wb = np.sign(w).astype(np.float32)  # (O, C, 3, 3)
    wbt = np.ascontiguousarray(
        wb.reshape(O, KT, 128, 9).transpose(2, 1, 3, 0)
    )  # (128, KT, 9, O)
    gamma2 = np.ascontiguousarray(gamma.reshape(KT, 128).T)  # (128, KT)
    beta2 = np.ascontiguousarray(beta.reshape(KT, 128).T)
    bvec = np.ascontiguousarray(b.reshape(O, 1))

    in_maps = []
    for i in range(NCORES):
        in_maps.append(
            {
                "x": np.ascontiguousarray(xp[i * NPER : (i + 1) * NPER]),
                "wbt": wbt,
                "gamma2": gamma2,
                "beta2": beta2,
                "bvec": bvec,
            }
        )
    return x, in_maps


_PROGRAM_CACHE: dict[str, bacc.Bacc] = {}


def get_program(variant: str | None = None) -> bacc.Bacc:
    if variant is None:
        variant = os.environ.get("BASS_VARIANT", "coltile")
    if variant not in _PROGRAM_CACHE:
        _PROGRAM_CACHE[variant] = build_program(variant)
    return _PROGRAM_CACHE[variant]


def run(inputs: dict, trace: bool = False, variant: str | None = None):
    """Returns (full_output, BassKernelResults)."""
    x, in_maps = prep_inputs(**inputs)
    nc = get_program(variant)
    res = run_bass_kernel_spmd(
        nc, in_maps, list(range(NCORES)), trace=trace
    )
    conv = np.concatenate(
        [np.asarray(res.results[i]["out"]) for i in range(NCORES)], axis=0
    )  # (32, 64, 56, 56)
    out = np.concatenate([x, conv], axis=1)  # (32, 320, 56, 56)
    return out, res


def kernel(**inputs) -> np.ndarray:
    out, _ = run(inputs)
    return out


# revision 17
# speedup vs baseline: 1.7665x; 1.7665x over previous
"""Trainium2 Bass kernel for DenseBlock: sync-BN (training stats) + binarized
3x3 conv + dense concat.

Reference computation (shapes hardcoded):
  x: (32, 256, 56, 56) f32
  mean/var over (N,H,W) per channel  ->  xn = (x-mean)*rsqrt(var+eps)*gamma+beta
  out_conv = conv3x3(xn, sign(w)) + b      (padding=1)
  return concat([x, out_conv], axis=1)     -> (32, 320, 56, 56)

Distribution: data-parallel over batch (4 images per core, 8 cores),
weights replicated, sync-BN via an on-device AllReduce of per-core
(sum, sumsq) partials.

Device layout per core:
  - x is host-padded to W=64 (cols 56..63 zero) so each row is a 64-element
    stride; each (ktile, image) lives in SBUF as [128p, 60, 64]: rows 0-1 and
    58-59 are zero padding, the image occupies rows 2..57. With this layout
    every 3x3 tap's input window is the SAME [8, 56] pattern shifted by
    dh*64 + dw elements, always reading in-bounds (pad rows/cols supply the
    conv zero padding exactly).
  - bn_stats/bn_aggr one-pass stats over the image cols 0..55 ->
    (sum, sumsq) -> 2KB AllReduce -> per-channel scale s, shift t
  - xn = s*x + t in place on image cols (kt0 on ACT, kt1 on DVE)
  - conv: per output tile (image n, 8-row block) the 9 taps x 2 K-tiles are
    18 matmuls, each writing the full [64, 8, 56] psum footprint (uniform
    accumulation group). The two K-tiles (C=256 -> 2x128) run CONCURRENTLY
    in the two 64-column halves of the PE array (col-tiling, M=64 each),
    psum partitions [0:64] / [64:128].
  - epilogue: out = (psum_lo + b) + psum_hi in one DVE op, DMA out
  - host concatenates raw x with the gathered conv outputs
"""

import os
import sys
from contextlib import ExitStack

import numpy as np

sys.path.insert(0, "/opt/trn_rl_repo")

from concourse import bacc, bass, mybir, tile  # noqa: E402
from concourse.bass_utils import run_bass_kernel_spmd  # noqa: E402

N, C, H, W, O = 32, 256, 56, 56, 64
NCORES = 8
NPER = N // NCORES  # 4 images per core
KT = 2  # channel tiles of 128
PIX = H * W  # 3136
EPS = 1e-5
HB = 8  # psum tile height (8 rows x 56 = 448 <= 512 f32 psum bank)
WP = 64  # host-padded row width
NHB = H // HB  # 7
TOP = 2  # top pad rows in the sbuf tile
ROWS = TOP + H + 2  # 60
F32 = mybir.dt.float32
BF16 = mybir.dt.bfloat16

TAPS = [(dh, dw) for dh in (-1, 0, 1) for dw in (-1, 0, 1)]


def bf16_hi_view(tile_ap, r0: int, c0: int, nrows: int, ncols: int):
    """A [128, nrows, ncols] bf16 access pattern over the HIGH 2 bytes of
    each f32 slot of a [128, ROWS, WP] f32 tile, starting at (r0, c0).
    f32 -> bf16 is the top 2 bytes (little-endian offset +1), so the f32
    zero padding reads as bf16 zero with no extra work."""
    tb = tile_ap.bitcast(BF16)  # [128, ROWS, 2*WP]
    return bass.AP(
        tensor=tb.tensor,
        offset=tb.offset + 1 + r0 * 2 * WP + c0 * 2,
        ap=[[tb.ap[0][0], 128], [2 * WP, nrows], [2, ncols]],
    )


def build_program(variant: str | None = None) -> bacc.Bacc:
    """variant: 'coltile' (default) runs the two K-tiles concurrently in the
    two column halves of the PE array; 'serial' accumulates all 18 matmuls
    into one [64, ...] psum tile."""
    if variant is None:
        variant = os.environ.get("BASS_VARIANT", "coltile")
    coltile = variant == "coltile"

    nc = bacc.Bacc(num_devices=NCORES)
    x_ext = nc.declare_dram_parameter("x", [NPER, C, ROWS, WP], F32, isOutput=False)
    w_ext = nc.declare_dram_parameter("wbt", [128, KT, 9, O], BF16, isOutput=False)
    g_ext = nc.declare_dram_parameter("gamma2", [128, KT], F32, isOutput=False)
    be_ext = nc.declare_dram_parameter("beta2", [128, KT], F32, isOutput=False)
    b_ext = nc.declare_dram_parameter("bvec", [O, 1], F32, isOutput=False)
    out_ext = nc.declare_dram_parameter("out", [NPER, O, H, W], F32, isOutput=True)

    with tile.TileContext(nc) as tc, ExitStack() as ctx:
        xpool = ctx.enter_context(tc.tile_pool(name="x", bufs=1))
        cpool = ctx.enter_context(tc.tile_pool(name="consts", bufs=1))
        spool = ctx.enter_context(tc.tile_pool(name="stats", bufs=1))
        pspool = ctx.enter_context(
            tc.tile_pool(name="psum", bufs=4, space=bass.MemorySpace.PSUM)
        )
        opool = ctx.enter_context(tc.tile_pool(name="ob", bufs=4))
        dpool = ctx.enter_context(tc.tile_pool(name="dram", bufs=1, space="DRAM"))

        # x shard: one tile per (channel-tile, image); image rows at [2:58]
        xk = [
            [xpool.tile([128, ROWS, WP], F32, tag=f"xk{k}_{n}", name=f"xk{k}_{n}")
             for n in range(NPER)]
            for k in range(KT)
        ]
        w_sb = cpool.tile([128, KT, 9, O], BF16, tag="w", name="w_sb")
        g_sb = cpool.tile([128, KT], F32, tag="g", name="g_sb")
        be_sb = cpool.tile([128, KT], F32, tag="be", name="be_sb")
        b_sb = cpool.tile([O, 1], F32, tag="b", name="b_sb")

        nc.sync.dma_start(out=w_sb[:], in_=w_ext[:])
        nc.sync.dma_start(out=g_sb[:], in_=g_ext[:])
        nc.sync.dma_start(out=be_sb[:], in_=be_ext[:])
        nc.sync.dma_start(out=b_sb[:], in_=b_ext[:])

        # all padding (rows AND cols) is baked into the host-side array, so
        # the tile's only producers are this DMA and the f32r xn ops
        for k in range(KT):
            for n in range(NPER):
                t = xk[k][n]
                nc.sync.dma_start(
                    out=t[:], in_=x_ext[n, k * 128 : (k + 1) * 128]
                )

        # ---- local batch-norm stats (one pass over x on DVE) ----
        # stats run over the full 64-wide rows INCLUDING the zero pad cols:
        # zeros add nothing to sum/sumsq, so converting (mean, var) with the
        # padded count recovers the exact partial sums.
        bns = spool.tile([128, KT, NPER, NHB, 6], F32, tag="bns", name="bns")
        for k in range(KT):
            for n in range(NPER):
                for ib in range(NHB):
                    r0 = TOP + ib * HB
                    nc.vector.bn_stats(
                        out=bns[:, k, n, ib],
                        in_=xk[k][n][:, r0 : r0 + HB, :].opt(),
                    )
        mv = spool.tile([128, KT, 2], F32, tag="mv", name="mv")  # (mean, var) local
        for k in range(KT):
            nc.vector.bn_aggr(out=mv[:, k], in_=bns[:, k])

        # convert to (sum, sumsq) partials for the cross-core reduction
        part = spool.tile([128, KT, 2], F32, tag="part", name="part")
        tmp = spool.tile([128, KT], F32, tag="tmp", name="tmp")
        npix = float(NPER * H * WP)  # padded count (zeros cancel)
        nc.vector.tensor_scalar_mul(part[:, :, 0], mv[:, :, 0], npix)
        nc.vector.tensor_mul(tmp[:], mv[:, :, 0], mv[:, :, 0])
        nc.vector.tensor_add(tmp[:], tmp[:], mv[:, :, 1])
        nc.vector.tensor_scalar_mul(part[:, :, 1], tmp[:], npix)

        cc_in = dpool.tile([128, KT, 2], F32, tag="ccin", name="cc_in")
        cc_out = dpool.tile(
            [128, KT, 2], F32, tag="ccout", name="cc_out", addr_space="Shared"
        )
        nc.gpsimd.dma_start(out=cc_in[:], in_=part[:])
        nc.gpsimd.collective_compute(
            "AllReduce",
            mybir.AluOpType.add,
            replica_groups=[list(range(NCORES))],
            ins=[cc_in[:].opt()],
            outs=[cc_out[:].opt()],
        )
        gpart = spool.tile([128, KT, 2], F32, tag="gpart", name="gpart")
        nc.gpsimd.dma_start(out=gpart[:], in_=cc_out[:])

        # ---- global scale/shift: s = gamma*rsqrt(var+eps), t = beta - mean*s
        gm = spool.tile([128, KT], F32, tag="gm", name="gm")
        vr = spool.tile([128, KT], F32, tag="vr", name="vr")
        msq = spool.tile([128, KT], F32, tag="msq", name="msq")
        s_sb = spool.tile([128, KT], F32, tag="s", name="s_sb")
        t_sb = spool.tile([128, KT], F32, tag="t", name="t_sb")
        inv_total = 1.0 / float(N * PIX)
        nc.vector.tensor_scalar_mul(gm[:], gpart[:, :, 0], inv_total)
        nc.vector.tensor_scalar_mul(vr[:], gpart[:, :, 1], inv_total)  # E[x^2]
        nc.vector.tensor_mul(msq[:], gm[:], gm[:])
        nc.vector.tensor_sub(vr[:], vr[:], msq[:])  # var
        epst = spool.tile([128, 1], F32, tag="eps", name="epst")
        nc.vector.memset(epst[:], EPS)
        nc.scalar.activation(
            vr[:], vr[:], mybir.ActivationFunctionType.Sqrt, bias=epst[:]
        )  # std
        nc.vector.reciprocal(vr[:], vr[:])  # 1/std
        nc.vector.tensor_mul(s_sb[:], g_sb[:], vr[:])
        nc.vector.tensor_mul(t_sb[:], gm[:], s_sb[:])
        nc.vector.tensor_sub(t_sb[:], be_sb[:], t_sb[:])

        # ---- xn = s*x + t in place on image cols; kt0 on ACT, kt1 on DVE
        for n in range(NPER):
            img0 = xk[0][n][:, TOP : TOP + H, 0:W]
            img1 = xk[1][n][:, TOP : TOP + H, 0:W]
            nc.scalar.activation(
                bf16_hi_view(xk[0][n][:], TOP, 0, H, W),
                img0,
                mybir.ActivationFunctionType.Identity,
                bias=t_sb[:, 0:1],
                scale=s_sb[:, 0:1],
            )
            nc.vector.tensor_scalar(
                bf16_hi_view(xk[1][n][:], TOP, 0, H, W),
                img1,
                s_sb[:, 1:2],
                t_sb[:, 1:2],
                mybir.AluOpType.mult,
                mybir.AluOpType.add,
            )

        # ---- conv: 18 uniform matmuls per output tile ----
        # rhs for tap (dh, dw) = the [8, 56] window shifted dh*64+dw elements
        for n in range(NPER):
            for ib in range(NHB):
                r0 = TOP + ib * HB
                if coltile:
                    ps = pspool.tile([128, HB, W], F32, tag="ps", name="ps")
                else:
                    ps = pspool.tile([O, HB, W], F32, tag="ps", name="ps")
                for ti, (dh, dw) in enumerate(TAPS):
                    tap = (dh + 1) * 3 + (dw + 1)
                    for k in range(KT):
                        if coltile:
                            out_ap = ps[64 * k : 64 * k + 64]
                            start = ti == 0
                            stop = ti == len(TAPS) - 1
                        else:
                            out_ap = ps[:]
                            start = ti == 0 and k == 0
                            stop = ti == len(TAPS) - 1 and k == KT - 1
                        # bf16 moving operand: single-pass full-rate matmul
                        # (fp32 runs as 2 half-rate LOW/HIGH passes)
                        nc.tensor.matmul(
                            out_ap,
                            w_sb[:, k, tap, :],
                            bf16_hi_view(xk[k][n][:], r0 + dh, dw, HB, W),
                            start=start,
                            stop=stop,
                            # the interp's group-conflict check is partition-
                            # blind; the two col-split halves falsely collide
                            skip_group_check=coltile,
                        )
                ob = opool.tile([O, HB, W], F32, tag="ob", name="ob")
                if coltile:
                    # PSUM reads may cross partitions (SB operands may not):
                    # ACT: ob_hi = psum_hi + b ; DVE: ob = ob_hi + psum_lo
                    ob_hi = opool.tile([O, HB, W], F32, tag="obhi", name="ob_hi")
                    nc.scalar.activation(
                        ob_hi[:],
                        ps[64:128],
                        mybir.ActivationFunctionType.Identity,
                        bias=b_sb[:],
                    )
                    nc.vector.tensor_add(ob[:], ob_hi[:], ps[0:64])
                else:
                    nc.vector.tensor_scalar_add(ob[:], ps[:], b_sb[:])
                nc.sync.dma_start(
                    out=out_ext[n, :, ib * HB : (ib + 1) * HB, :], in_=ob[:]
                )

    nc.finalize()
    return nc


def prep_inputs(x, gamma, beta, w, b):
    """Host-side layout prep. Returns (raw x, per-core input maps)."""
    x = np.ascontiguousarray(np.asarray(x, dtype=np.float32))
    gamma = np.asarray(gamma, dtype=np.float32)
    beta = np.asarray(beta, dtype=np.float32)
    w = np.asarray(w, dtype=np.float32)
    b = np.asarray(b, dtype=np.float32)

    # bake the conv zero padding into the array: 2 zero rows top, 2 bottom,
    # zero cols 56..63 (rows at [2:58], cols at [0:56])
    xp = np.zeros((N, C, TOP + H + 2, WP), dtype=np.float32)
    xp[:, :, TOP : TOP + H, :W] = x

    # sign(w) transposed to [c_local=128, kt, tap, o], contiguous
    import ml_dtypes

    wb = np.sign(w).astype(np.float32)  # (O, C, 3, 3)
    wbt = np.ascontiguousarray(
        wb.reshape(O, KT, 128, 9).transpose(2, 1, 3, 0).astype(ml_dtypes.bfloat16)
    )  # (128, KT, 9, O) bf16; sign values are exact in bf16
    gamma2 = np.ascontiguousarray(gamma.reshape(KT, 128).T)  # (128, KT)
    beta2 = np.ascontiguousarray(beta.reshape(KT, 128).T)
    bvec = np.ascontiguousarray(b.reshape(O, 1))

    in_maps = []
    for i in range(NCORES):
        in_maps.append(
            {
                "x": np.ascontiguousarray(xp[i * NPER : (i + 1) * NPER]),
                "wbt": wbt,
                "gamma2": gamma2,
                "beta2": beta2,
                "bvec": bvec,
            }
        )
    return x, in_maps


_PROGRAM_CACHE: dict[str, bacc.Bacc] = {}


def get_program(variant: str | None = None) -> bacc.Bacc:
    if variant is None:
        variant = os.environ.get("BASS_VARIANT", "coltile")
    if variant not in _PROGRAM_CACHE:
        _PROGRAM_CACHE[variant] = build_program(variant)
    return _PROGRAM_CACHE[variant]


def run(inputs: dict, trace: bool = False, variant: str | None = None):
    """Returns (full_output, BassKernelResults)."""
    x, in_maps = prep_inputs(**inputs)
    nc = get_program(variant)
    res = run_bass_kernel_spmd(
        nc, in_maps, list(range(NCORES)), trace=trace
    )
    conv = np.concatenate(
        [np.asarray(res.results[i]["out"]) for i in range(NCORES)], axis=0
    )  # (32, 64, 56, 56)
    out = np.concatenate([x, conv], axis=1)  # (32, 320, 56, 56)
    return out, res


def kernel(**inputs) -> np.ndarray:
    out, _ = run(inputs)
    return out


# revision 18
# speedup vs baseline: 2.1348x; 1.2085x over previous
"""Trainium2 Bass kernel for DenseBlock: sync-BN (training stats) + binarized
3x3 conv + dense concat.

Reference computation (shapes hardcoded):
  x: (32, 256, 56, 56) f32
  mean/var over (N,H,W) per channel  ->  xn = (x-mean)*rsqrt(var+eps)*gamma+beta
  out_conv = conv3x3(xn, sign(w)) + b      (padding=1)
  return concat([x, out_conv], axis=1)     -> (32, 320, 56, 56)

Distribution: data-parallel over batch (4 images per core, 8 cores),
weights replicated, sync-BN via an on-device AllReduce of per-core
(sum, sumsq) partials.

Device layout per core:
  - x is host-padded to W=64 (cols 56..63 zero) so each row is a 64-element
    stride; each (ktile, image) lives in SBUF as [128p, 60, 64]: rows 0-1 and
    58-59 are zero padding, the image occupies rows 2..57. With this layout
    every 3x3 tap's input window is the SAME [8, 56] pattern shifted by
    dh*64 + dw elements, always reading in-bounds (pad rows/cols supply the
    conv zero padding exactly).
  - bn_stats/bn_aggr one-pass stats over the image cols 0..55 ->
    (sum, sumsq) -> 2KB AllReduce -> per-channel scale s, shift t
  - xn = s*x + t in place on image cols (kt0 on ACT, kt1 on DVE)
  - conv: per output tile (image n, 8-row block) the 9 taps x 2 K-tiles are
    18 matmuls, each writing the full [64, 8, 56] psum footprint (uniform
    accumulation group). The two K-tiles (C=256 -> 2x128) run CONCURRENTLY
    in the two 64-column halves of the PE array (col-tiling, M=64 each),
    psum partitions [0:64] / [64:128].
  - epilogue: out = (psum_lo + b) + psum_hi in one DVE op, DMA out
  - host concatenates raw x with the gathered conv outputs
"""

import os
import sys
from contextlib import ExitStack

import numpy as np

sys.path.insert(0, "/opt/trn_rl_repo")

from concourse import bacc, bass, mybir, tile  # noqa: E402
from concourse.bass_utils import run_bass_kernel_spmd  # noqa: E402

N, C, H, W, O = 32, 256, 56, 56, 64
NCORES = 8
NPER = N // NCORES  # 4 images per core
KT = 2  # channel tiles of 128
PIX = H * W  # 3136
EPS = 1e-5
HB = 8  # psum tile height (8 rows x 56 = 448 <= 512 f32 psum bank)
WP = 64  # host-padded row width
NHB = H // HB  # 7
TOP = 2  # top pad rows in the sbuf tile
ROWS = TOP + H + 2  # 60
F32 = mybir.dt.float32
BF16 = mybir.dt.bfloat16

TAPS = [(dh, dw) for dh in (-1, 0, 1) for dw in (-1, 0, 1)]


def bf16_window(tile_ap, r0: int, c0: int, nrows: int, ncols: int):
    """A [128, nrows, ncols] window of a [128, ROWS, WP] bf16 tile at
    (r0, c0); c0 may be -1 (reads the previous row's zero pad col)."""
    return bass.AP(
        tensor=tile_ap.tensor,
        offset=tile_ap.offset + r0 * WP + c0,
        ap=[[tile_ap.ap[0][0], 128], [WP, nrows], [1, ncols]],
    )


def build_program(variant: str | None = None) -> bacc.Bacc:
    """variant: 'coltile' (default) runs the two K-tiles concurrently in the
    two column halves of the PE array; 'serial' accumulates all 18 matmuls
    into one [64, ...] psum tile."""
    if variant is None:
        variant = os.environ.get("BASS_VARIANT", "coltile")
    coltile = variant == "coltile"

    nc = bacc.Bacc(num_devices=NCORES)
    x_ext = nc.declare_dram_parameter("x", [NPER, C, ROWS, WP], BF16, isOutput=False)
    w_ext = nc.declare_dram_parameter("wbt", [128, KT, 9, O], BF16, isOutput=False)
    g_ext = nc.declare_dram_parameter("gamma2", [128, KT], F32, isOutput=False)
    be_ext = nc.declare_dram_parameter("beta2", [128, KT], F32, isOutput=False)
    b_ext = nc.declare_dram_parameter("bvec", [O, 1], F32, isOutput=False)
    out_ext = nc.declare_dram_parameter("out", [NPER, O, H, W], F32, isOutput=True)

    with tile.TileContext(nc) as tc, ExitStack() as ctx:
        xpool = ctx.enter_context(tc.tile_pool(name="x", bufs=1))
        cpool = ctx.enter_context(tc.tile_pool(name="consts", bufs=1))
        spool = ctx.enter_context(tc.tile_pool(name="stats", bufs=1))
        pspool = ctx.enter_context(
            tc.tile_pool(name="psum", bufs=4, space=bass.MemorySpace.PSUM)
        )
        opool = ctx.enter_context(tc.tile_pool(name="ob", bufs=4))
        dpool = ctx.enter_context(tc.tile_pool(name="dram", bufs=1, space="DRAM"))

        # x shard: one tile per (channel-tile, image); image rows at [2:58]
        xk = [
            [xpool.tile([128, ROWS, WP], BF16, tag=f"xk{k}_{n}", name=f"xk{k}_{n}")
             for n in range(NPER)]
            for k in range(KT)
        ]
        w_sb = cpool.tile([128, KT, 9, O], BF16, tag="w", name="w_sb")
        g_sb = cpool.tile([128, KT], F32, tag="g", name="g_sb")
        be_sb = cpool.tile([128, KT], F32, tag="be", name="be_sb")
        b_sb = cpool.tile([O, 1], F32, tag="b", name="b_sb")

        nc.sync.dma_start(out=w_sb[:], in_=w_ext[:])
        nc.sync.dma_start(out=g_sb[:], in_=g_ext[:])
        nc.sync.dma_start(out=be_sb[:], in_=be_ext[:])
        nc.sync.dma_start(out=b_sb[:], in_=b_ext[:])

        # all padding (rows AND cols) is baked into the host-side array.
        # chunked loads so bn_stats can start on early rows while later
        # rows are still in flight
        RC = 15  # row chunk
        for k in range(KT):
            for n in range(NPER):
                t = xk[k][n]
                for r in range(0, ROWS, RC):
                    r1 = min(r + RC, ROWS)
                    nc.sync.dma_start(
                        out=t[:, r:r1, :],
                        in_=x_ext[n, k * 128 : (k + 1) * 128, r:r1, :],
                    )

        # ---- local batch-norm stats (one pass over x on DVE) ----
        # stats run over the full 64-wide rows INCLUDING the zero pad cols:
        # zeros add nothing to sum/sumsq, so converting (mean, var) with the
        # padded count recovers the exact partial sums.
        bns = spool.tile([128, KT, NPER, NHB, 6], F32, tag="bns", name="bns")
        for k in range(KT):
            for n in range(NPER):
                for ib in range(NHB):
                    r0 = TOP + ib * HB
                    nc.vector.bn_stats(
                        out=bns[:, k, n, ib],
                        in_=xk[k][n][:, r0 : r0 + HB, :].opt(),
                    )
        mv = spool.tile([128, KT, 2], F32, tag="mv", name="mv")  # (mean, var) local
        for k in range(KT):
            nc.vector.bn_aggr(out=mv[:, k], in_=bns[:, k])

        # convert to (sum, sumsq) partials for the cross-core reduction
        part = spool.tile([128, KT, 2], F32, tag="part", name="part")
        tmp = spool.tile([128, KT], F32, tag="tmp", name="tmp")
        npix = float(NPER * H * WP)  # padded count (zeros cancel)
        nc.vector.tensor_scalar_mul(part[:, :, 0], mv[:, :, 0], npix)
        nc.vector.tensor_mul(tmp[:], mv[:, :, 0], mv[:, :, 0])
        nc.vector.tensor_add(tmp[:], tmp[:], mv[:, :, 1])
        nc.vector.tensor_scalar_mul(part[:, :, 1], tmp[:], npix)

        cc_in = dpool.tile([128, KT, 2], F32, tag="ccin", name="cc_in")
        cc_out = dpool.tile(
            [128, KT, 2], F32, tag="ccout", name="cc_out", addr_space="Shared"
        )
        nc.gpsimd.dma_start(out=cc_in[:], in_=part[:])
        nc.gpsimd.collective_compute(
            "AllReduce",
            mybir.AluOpType.add,
            replica_groups=[list(range(NCORES))],
            ins=[cc_in[:].opt()],
            outs=[cc_out[:].opt()],
        )
        gpart = spool.tile([128, KT, 2], F32, tag="gpart", name="gpart")
        nc.gpsimd.dma_start(out=gpart[:], in_=cc_out[:])

        # ---- global scale/shift: s = gamma*rsqrt(var+eps), t = beta - mean*s
        gm = spool.tile([128, KT], F32, tag="gm", name="gm")
        vr = spool.tile([128, KT], F32, tag="vr", name="vr")
        msq = spool.tile([128, KT], F32, tag="msq", name="msq")
        s_sb = spool.tile([128, KT], F32, tag="s", name="s_sb")
        t_sb = spool.tile([128, KT], F32, tag="t", name="t_sb")
        inv_total = 1.0 / float(N * PIX)
        nc.vector.tensor_scalar_mul(gm[:], gpart[:, :, 0], inv_total)
        nc.vector.tensor_scalar_mul(vr[:], gpart[:, :, 1], inv_total)  # E[x^2]
        nc.vector.tensor_mul(msq[:], gm[:], gm[:])
        nc.vector.tensor_sub(vr[:], vr[:], msq[:])  # var
        epst = spool.tile([128, 1], F32, tag="eps", name="epst")
        nc.vector.memset(epst[:], EPS)
        nc.scalar.activation(
            vr[:], vr[:], mybir.ActivationFunctionType.Sqrt, bias=epst[:]
        )  # std
        nc.vector.reciprocal(vr[:], vr[:])  # 1/std
        nc.vector.tensor_mul(s_sb[:], g_sb[:], vr[:])
        nc.vector.tensor_mul(t_sb[:], gm[:], s_sb[:])
        nc.vector.tensor_sub(t_sb[:], be_sb[:], t_sb[:])

        # ---- xn = s*x + t in place on image cols; kt0 on ACT, kt1 on DVE
        for n in range(NPER):
            img0 = xk[0][n][:, TOP : TOP + H, 0:W]
            img1 = xk[1][n][:, TOP : TOP + H, 0:W]
            nc.scalar.activation(
                bf16_window(xk[0][n][:], TOP, 0, H, W),
                img0,
                mybir.ActivationFunctionType.Identity,
                bias=t_sb[:, 0:1],
                scale=s_sb[:, 0:1],
            )
            nc.vector.tensor_scalar(
                bf16_window(xk[1][n][:], TOP, 0, H, W),
                img1,
                s_sb[:, 1:2],
                t_sb[:, 1:2],
                mybir.AluOpType.mult,
                mybir.AluOpType.add,
            )

        # ---- conv: 18 uniform matmuls per output tile ----
        # rhs for tap (dh, dw) = the [8, 56] window shifted dh*64+dw elements
        for n in range(NPER):
            for ib in range(NHB):
                r0 = TOP + ib * HB
                if coltile:
                    ps = pspool.tile([128, HB, W], F32, tag="ps", name="ps")
                else:
                    ps = pspool.tile([O, HB, W], F32, tag="ps", name="ps")
                for ti, (dh, dw) in enumerate(TAPS):
                    tap = (dh + 1) * 3 + (dw + 1)
                    for k in range(KT):
                        if coltile:
                            out_ap = ps[64 * k : 64 * k + 64]
                            start = ti == 0
                            stop = ti == len(TAPS) - 1
                        else:
                            out_ap = ps[:]
                            start = ti == 0 and k == 0
                            stop = ti == len(TAPS) - 1 and k == KT - 1
                        # bf16 moving operand: single-pass full-rate matmul
                        # (fp32 runs as 2 half-rate LOW/HIGH passes)
                        nc.tensor.matmul(
                            out_ap,
                            w_sb[:, k, tap, :],
                            bf16_window(xk[k][n][:], r0 + dh, dw, HB, W),
                            start=start,
                            stop=stop,
                            # the interp's group-conflict check is partition-
                            # blind; the two col-split halves falsely collide
                            skip_group_check=coltile,
                        )
                ob = opool.tile([O, HB, W], F32, tag="ob", name="ob")
                if coltile:
                    # PSUM reads may cross partitions (SB operands may not):
                    # ACT: ob_hi = psum_hi + b ; DVE: ob = ob_hi + psum_lo
                    ob_hi = opool.tile([O, HB, W], F32, tag="obhi", name="ob_hi")
                    nc.scalar.activation(
                        ob_hi[:],
                        ps[64:128],
                        mybir.ActivationFunctionType.Identity,
                        bias=b_sb[:],
                    )
                    nc.vector.tensor_add(ob[:], ob_hi[:], ps[0:64])
                else:
                    nc.vector.tensor_scalar_add(ob[:], ps[:], b_sb[:])
                nc.sync.dma_start(
                    out=out_ext[n, :, ib * HB : (ib + 1) * HB, :], in_=ob[:]
                )

    nc.finalize()
    return nc


def prep_inputs(x, gamma, beta, w, b):
    """Host-side layout prep. Returns (raw x, per-core input maps)."""
    x = np.ascontiguousarray(np.asarray(x, dtype=np.float32))
    gamma = np.asarray(gamma, dtype=np.float32)
    beta = np.asarray(beta, dtype=np.float32)
    w = np.asarray(w, dtype=np.float32)
    b = np.asarray(b, dtype=np.float32)

    import ml_dtypes

    # bake the conv zero padding into the array: 2 zero rows top, 2 bottom,
    # zero cols 56..63 (rows at [2:58], cols at [0:56]); bf16 storage
    xp = np.zeros((N, C, TOP + H + 2, WP), dtype=ml_dtypes.bfloat16)
    xp[:, :, TOP : TOP + H, :W] = x.astype(ml_dtypes.bfloat16)

    # sign(w) transposed to [c_local=128, kt, tap, o], contiguous
    wb = np.sign(w).astype(np.float32)  # (O, C, 3, 3)
    wbt = np.ascontiguousarray(
        wb.reshape(O, KT, 128, 9).transpose(2, 1, 3, 0).astype(ml_dtypes.bfloat16)
    )  # (128, KT, 9, O) bf16; sign values are exact in bf16
    gamma2 = np.ascontiguousarray(gamma.reshape(KT, 128).T)  # (128, KT)
    beta2 = np.ascontiguousarray(beta.reshape(KT, 128).T)
    bvec = np.ascontiguousarray(b.reshape(O, 1))

    in_maps = []
    for i in range(NCORES):
        in_maps.append(
            {
                "x": np.ascontiguousarray(xp[i * NPER : (i + 1) * NPER]),
                "wbt": wbt,
                "gamma2": gamma2,
                "beta2": beta2,
                "bvec": bvec,
            }
        )
    return x, in_maps


_PROGRAM_CACHE: dict[str, bacc.Bacc] = {}


def get_program(variant: str | None = None) -> bacc.Bacc:
    if variant is None:
        variant = os.environ.get("BASS_VARIANT", "coltile")
    if variant not in _PROGRAM_CACHE:
        _PROGRAM_CACHE[variant] = build_program(variant)
    return _PROGRAM_CACHE[variant]


def run(inputs: dict, trace: bool = False, variant: str | None = None):
    """Returns (full_output, BassKernelResults)."""
    x, in_maps = prep_inputs(**inputs)
    nc = get_program(variant)
    res = run_bass_kernel_spmd(
        nc, in_maps, list(range(NCORES)), trace=trace
    )
    conv = np.concatenate(
        [np.asarray(res.results[i]["out"]) for i in range(NCORES)], axis=0
    )  # (32, 64, 56, 56)
    out = np.concatenate([x, conv], axis=1)  # (32, 320, 56, 56)
    return out, res


def kernel(**inputs) -> np.ndarray:
    out, _ = run(inputs)
    return out
